# revision 5
# baseline (speedup 1.0000x reference)
"""Trainium2 Bass kernel for nn_CCE_Head (B=8, C=512, N=19, H=W=128).

Data-parallel over batch: one sample per NeuronCore (8 cores).
Self-contained: hardcodes shapes; builds + compiles a Bass module once,
then runs it SPMD on cores 0-7 via run_bass_kernel_spmd.
"""

import numpy as np

import concourse.bacc as bacc
import concourse.bass as bass
import concourse.tile as tile
from concourse import mybir
from concourse import bass_utils
from concourse.masks import make_identity

F32 = mybir.dt.float32
BF16 = mybir.dt.bfloat16
AF = mybir.ActivationFunctionType
ALU = mybir.AluOpType

B, C, N, H, W = 8, 512, 19, 128, 128
HW = H * W                      # 16384
CB = C // 128                   # 4 c-tiles
NCH = HW // 128                 # 128 p-chunks of width 128
GRP = 8                         # chunks per group (psum bank limit: 8*60*4B=1920)
NGRP = NCH // GRP               # 16 groups
FSLOT = 20                      # per-field column slot (19 used + 1 pad)
LN_EPS = 1e-5


def build_module(repeat=1):
    nc = bacc.Bacc("TRN2", target_bir_lowering=False, debug=False,
                   enable_asserts=False)

    x_d = nc.dram_tensor("x", [C, HW], F32, kind="ExternalInput")
    wt_d = nc.dram_tensor("wt60", [C, 3 * FSLOT], F32, kind="ExternalInput")
    bias_d = nc.dram_tensor("bias60", [3 * FSLOT], F32, kind="ExternalInput")
    maskw_d = nc.dram_tensor("maskw", [C], F32, kind="ExternalInput")
    cm1t_d = nc.dram_tensor("cm1T", [C, C], F32, kind="ExternalInput")
    cm1b_d = nc.dram_tensor("cm1b", [C], F32, kind="ExternalInput")
    lng_d = nc.dram_tensor("lng", [C], F32, kind="ExternalInput")
    lnb_d = nc.dram_tensor("lnb", [C], F32, kind="ExternalInput")
    cm2t_d = nc.dram_tensor("cm2T", [C, C], F32, kind="ExternalInput")
    cm2b_d = nc.dram_tensor("cm2b", [C], F32, kind="ExternalInput")
    finwt_d = nc.dram_tensor("finWT", [C, N], F32, kind="ExternalInput")
    finb_d = nc.dram_tensor("finb", [N], F32, kind="ExternalInput")
    out_d = nc.dram_tensor("out", [N, HW], F32, kind="ExternalOutput")

    def bcast_ap(handle, reps, inner):
        # [p-broadcast, rep-broadcast, inner] view of a 1-D dram tensor
        a = handle.ap()
        return bass.AP(tensor=a.tensor, offset=a.offset,
                       ap=[[0, 128], [0, reps], [1, inner]])

    with tile.TileContext(nc) as tc:
        import contextlib
        with contextlib.ExitStack() as ctx:
            const = ctx.enter_context(tc.tile_pool(name="const", bufs=1))
            psum_keep = ctx.enter_context(
                tc.tile_pool(name="psum_keep", bufs=1, space="PSUM"))

            # ---------------- constants / weights ----------------
            ident = const.tile([128, 128], BF16)
            make_identity(nc, ident)

            wt_sb = const.tile([128, CB, 3 * FSLOT], BF16)
            nc.gpsimd.dma_start(
                out=wt_sb, in_=wt_d.ap().rearrange("(cb p) f -> p cb f", cb=CB))

            bias_grp = const.tile([128, GRP, 3 * FSLOT], F32)
            nc.gpsimd.dma_start(out=bias_grp, in_=bcast_ap(bias_d, GRP, 3 * FSLOT))

            maskw_sb = const.tile([N, C], F32)
            a = maskw_d.ap()
            nc.gpsimd.dma_start(
                out=maskw_sb,
                in_=bass.AP(tensor=a.tensor, offset=a.offset, ap=[[0, N], [1, C]]))

            cm1t_sb = const.tile([128, CB, C], F32)
            nc.sync.dma_start(
                out=cm1t_sb, in_=cm1t_d.ap().rearrange("(cb p) m -> p cb m", cb=CB))
            cm2t_sb = const.tile([128, CB, C], F32)
            nc.sync.dma_start(
                out=cm2t_sb, in_=cm2t_d.ap().rearrange("(mb p) c -> p mb c", mb=CB))
            finwt_sb = const.tile([128, CB, N], F32)
            nc.sync.dma_start(
                out=finwt_sb, in_=finwt_d.ap().rearrange("(cb p) n -> p cb n", cb=CB))

            def col128(handle):
                # [512] dram -> [128, 4] sbuf (m = mj*128 + p)
                t = const.tile([128, CB], F32, name=f"{handle.name}_sb")
                a = handle.ap()
                nc.gpsimd.dma_start(
                    out=t, in_=bass.AP(tensor=a.tensor, offset=a.offset,
                                       ap=[[1, 128], [128, CB]]))
                return t

            cm1b_sb = col128(cm1b_d)
            lng_sb = col128(lng_d)
            lnb_sb = col128(lnb_d)
            cm2b_sb = col128(cm2b_d)

            finb_sb = const.tile([N, 1], F32)
            a = finb_d.ap()
            nc.gpsimd.dma_start(
                out=finb_sb,
                in_=bass.AP(tensor=a.tensor, offset=a.offset, ap=[[1, N], [0, 1]]))

            ones_bf = const.tile([128, 1], BF16)
            nc.vector.memset(ones_bf, 1.0)
            ones_col = const.tile([128, 1], F32)
            nc.vector.memset(ones_col, 1.0)
            ones_row = const.tile([1, 128], F32)
            nc.vector.memset(ones_row, 1.0)
            one1 = const.tile([1, 1], F32)
            nc.vector.memset(one1, 1.0)
            ones19 = const.tile([N, 1], F32)
            nc.vector.memset(ones19, 1.0)
            half_bias = const.tile([128, 1], F32)
            nc.vector.memset(half_bias, 0.5)

            # pin exp_and_others as the first-loaded ACT table set
            dummy = const.tile([1, 1], F32)
            nc.vector.memset(dummy, 0.0)
            nc.scalar.activation(out=dummy, in_=dummy, func=AF.Exp)

            # ---------------- x load (f32 -> bf16 cast DMA) ----------------
            x_sb = const.tile([128, CB, HW], BF16)
            xr = x_d.ap().rearrange("(cb p) q -> p cb q", cb=CB)
            NSTRIPE = 8
            sw = HW // NSTRIPE
            for s in range(NSTRIPE):
                nc.gpsimd.dma_start(out=x_sb[:, :, s * sw:(s + 1) * sw],
                                    in_=xr[:, :, s * sw:(s + 1) * sw])

            # persistent accumulators
            psum_ocr = psum_keep.tile([N, C], F32)
            psum_sums = psum_keep.tile([1, GRP * N], F32)

            out_ring = ctx.enter_context(tc.tile_pool(name="out_ring", bufs=2))
            head_sb = ctx.enter_context(tc.tile_pool(name="head_sb", bufs=1))

            def main_body():
                with contextlib.ExitStack() as mctx:
                    grp_pool = mctx.enter_context(
                        tc.tile_pool(name="grp", bufs=2))
                    xt_pool = mctx.enter_context(
                        tc.tile_pool(name="xt", bufs=2))
                    psA_pool = mctx.enter_context(
                        tc.tile_pool(name="psA", bufs=2, space="PSUM"))
                    psB_pool = mctx.enter_context(
                        tc.tile_pool(name="psB", bufs=3, space="PSUM"))

                    for g in range(NGRP):
                        psA = psA_pool.tile([128, GRP, 3 * FSLOT], F32, name="psA")
                        grp = grp_pool.tile([128, GRP, 4 * FSLOT], BF16, name="grp")
                        xt = xt_pool.tile([128, GRP, C], BF16, name="xt")
                        for j in range(GRP):
                            chunk = g * GRP + j
                            p0 = chunk * 128
                            psB = psB_pool.tile([128, C], F32, name="psB")
                            for cb in range(CB):
                                lhsT = x_sb[:, cb, p0:p0 + 128]
                                nc.tensor.matmul(psA[:, j, :], lhsT, wt_sb[:, cb, :],
                                                 start=(cb == 0), stop=(cb == CB - 1))
                                nc.tensor.matmul(psB[:, cb * 128:(cb + 1) * 128],
                                                 lhsT, ident, start=True, stop=True)
                            # evacuate xT chunk (alternate engines)
                            if j % 2 == 0:
                                nc.scalar.copy(out=xt[:, j, :], in_=psB)
                            else:
                                nc.vector.tensor_copy(out=xt[:, j, :], in_=psB)

                        # mdbd + bias -> sbuf bf16 (m|d|bd in 20-col slots)
                        m_ = grp[:, :, 0:FSLOT]
                        d_ = grp[:, :, FSLOT:2 * FSLOT]
                        bd_ = grp[:, :, 2 * FSLOT:3 * FSLOT]
                        tmp = grp[:, :, 3 * FSLOT:4 * FSLOT]
                        nc.vector.tensor_add(grp[:, :, 0:3 * FSLOT], psA, bias_grp)

                        # sigmoid chain via tanh (exp_and_others set)
                        nc.scalar.activation(out=tmp, in_=m_, func=AF.Tanh, scale=0.5)
                        nc.vector.scalar_tensor_tensor(
                            out=d_, in0=tmp, scalar=1.0, in1=d_,
                            op0=ALU.add, op1=ALU.mult)          # v1 = 2*d1
                        nc.scalar.activation(out=tmp, in_=d_, func=AF.Tanh, scale=0.25)
                        nc.vector.scalar_tensor_tensor(
                            out=bd_, in0=tmp, scalar=1.0, in1=bd_,
                            op0=ALU.add, op1=ALU.mult)          # v2 = 2*bd1
                        nc.scalar.activation(out=tmp, in_=bd_, func=AF.Tanh, scale=0.25)
                        nc.vector.scalar_tensor_tensor(
                            out=d_, in0=tmp, scalar=1.0, in1=d_,
                            op0=ALU.add, op1=ALU.add)           # w = 2*d2
                        nc.scalar.activation(out=tmp, in_=d_, func=AF.Tanh, scale=0.25)
                        nc.vector.scalar_tensor_tensor(
                            out=m_, in0=tmp, scalar=0.5, in1=m_,
                            op0=ALU.mult, op1=ALU.add)          # p1 = probs - 0.5
                        nc.scalar.activation(out=m_, in_=m_, func=AF.Exp,
                                             bias=half_bias)    # e = exp(probs)

                        # softmax denominator partial sums
                        nc.tensor.matmul(psum_sums, ones_bf, grp[:, :, 0:N],
                                         start=(g == 0), stop=(g == NGRP - 1),
                                         skip_group_check=True)
                        # ocr accumulation
                        for j in range(GRP):
                            chunk = g * GRP + j
                            nc.tensor.matmul(psum_ocr, grp[:, j, 0:N], xt[:, j, :],
                                             start=(chunk == 0),
                                             stop=(chunk == NCH - 1),
                                             skip_group_check=True)

                # ---------------- head ----------------
                with contextlib.ExitStack() as hctx:
                    ph_pool = hctx.enter_context(
                        tc.tile_pool(name="ph", bufs=3, space="PSUM"))
                    pht_pool = hctx.enter_context(
                        tc.tile_pool(name="pht", bufs=1, space="PSUM"))
                    hs = head_sb

                    # softmax denominators -> [1, N] -> recip -> [N, 1]
                    sums_v = psum_sums.rearrange("o (j n) -> o n j", n=N)
                    rsum_row = hs.tile([1, N], F32, name="rsum_row")
                    nc.vector.tensor_reduce(out=rsum_row, in_=sums_v,
                                            axis=mybir.AxisListType.X, op=ALU.add)
                    nc.vector.reciprocal(out=rsum_row, in_=rsum_row)
                    ps_rsT = ph_pool.tile([N, 1], F32, name="ps_rsT", tag="ph")
                    nc.tensor.matmul(ps_rsT, rsum_row, one1, start=True, stop=True)
                    rsum_col = hs.tile([N, 1], F32, name="rsum_col")
                    nc.scalar.copy(out=rsum_col, in_=ps_rsT)

                    # normalized ocr  [N, C] f32
                    ocr_sb = hs.tile([N, C], F32, name="ocr_sb")
                    nc.vector.tensor_scalar(out=ocr_sb, in0=psum_ocr,
                                            scalar1=rsum_col, scalar2=None,
                                            op0=ALU.mult)

                    # attention over N
                    att_scr = hs.tile([N, C], F32, name="att_scr")
                    att_raw = hs.tile([N, 1], F32, name="att_raw")
                    nc.vector.scalar_tensor_tensor(
                        out=att_scr, in0=ocr_sb, scalar=1.0, in1=maskw_sb,
                        op0=ALU.mult, op1=ALU.mult, accum_out=att_raw)
                    eatt = hs.tile([N, 1], F32, name="eatt")
                    nc.scalar.activation(out=eatt, in_=att_raw, func=AF.Exp)
                    ps_s = ph_pool.tile([1, 1], F32, name="ps_s", tag="ph")
                    nc.tensor.matmul(ps_s, eatt, ones19, start=True, stop=True)
                    srecip = hs.tile([1, 1], F32, name="srecip")
                    nc.vector.reciprocal(out=srecip, in_=ps_s)

                    # ctx = (ocr^T @ eatt) / sum  -> [1, C] -> transpose [128, 4]
                    ps_ctx = ph_pool.tile([1, C], F32, name="ps_ctx", tag="ph")
                    nc.tensor.matmul(ps_ctx, eatt, ocr_sb, start=True, stop=True)
                    ctx_sb = hs.tile([1, C], F32, name="ctx_sb")
                    nc.vector.tensor_scalar(out=ctx_sb, in0=ps_ctx, scalar1=srecip,
                                            scalar2=None, op0=ALU.mult)
                    ps_ctxT = ph_pool.tile([128, CB], F32, name="ps_ctxT", tag="ph")
                    for cb in range(CB):
                        nc.tensor.matmul(ps_ctxT[:, cb:cb + 1],
                                         ctx_sb[0:1, cb * 128:(cb + 1) * 128], one1,
                                         start=True, stop=True)
                    ctxT_sb = hs.tile([128, CB], F32, name="ctxT_sb")
                    nc.scalar.copy(out=ctxT_sb, in_=ps_ctxT)

                    # t = cm1 @ ctx + b
                    ps_t = pht_pool.tile([128, CB], F32, name="ps_t", tag="pht")
                    for mj in range(CB):
                        for cb in range(CB):
                            nc.tensor.matmul(
                                ps_t[:, mj:mj + 1],
                                cm1t_sb[:, cb, mj * 128:(mj + 1) * 128],
                                ctxT_sb[:, cb:cb + 1],
                                start=(cb == 0), stop=(cb == CB - 1))
                    t_sb = hs.tile([128, CB], F32, name="t_sb")
                    nc.vector.tensor_add(t_sb, ps_t, cm1b_sb)

                    # layernorm stats over all 512
                    sq_sb = hs.tile([128, CB], F32, name="sq_sb")
                    nc.scalar.activation(out=sq_sb, in_=t_sb, func=AF.Square)
                    ps_st1 = ph_pool.tile([1, CB], F32, name="ps_st1", tag="ph")
                    nc.tensor.matmul(ps_st1, ones_col, t_sb, start=True, stop=True)
                    ps_st2 = ph_pool.tile([1, CB], F32, name="ps_st2", tag="ph")
                    nc.tensor.matmul(ps_st2, ones_col, sq_sb, start=True, stop=True)
                    s1 = hs.tile([1, 1], F32, name="s1")
                    nc.vector.tensor_reduce(out=s1, in_=ps_st1,
                                            axis=mybir.AxisListType.X, op=ALU.add)
                    s2 = hs.tile([1, 1], F32, name="s2")
                    nc.vector.tensor_reduce(out=s2, in_=ps_st2,
                                            axis=mybir.AxisListType.X, op=ALU.add)
                    mu = hs.tile([1, 1], F32, name="mu")
                    nc.vector.tensor_scalar(out=mu, in0=s1, scalar1=1.0 / C,
                                            scalar2=None, op0=ALU.mult)
                    ms2e = hs.tile([1, 1], F32, name="ms2e")
                    nc.vector.tensor_scalar(out=ms2e, in0=s2, scalar1=1.0 / C,
                                            scalar2=LN_EPS, op0=ALU.mult,
                                            op1=ALU.add)
                    # vpe = ms2e - mu^2 = var + eps
                    mu2 = hs.tile([1, 1], F32, name="mu2")
                    nc.vector.tensor_mul(mu2, mu, mu)
                    vpe = hs.tile([1, 1], F32, name="vpe")
                    nc.vector.tensor_sub(vpe, ms2e, mu2)
                    # rsq = exp(-0.5 * ln(var + eps))   (natural_log_exp set)
                    lnv = hs.tile([1, 1], F32, name="lnv")
                    nc.scalar.activation(out=lnv, in_=vpe, func=AF.Ln)
                    rsq = hs.tile([1, 1], F32, name="rsq")
                    nc.scalar.activation(out=rsq, in_=lnv, func=AF.Exp, scale=-0.5)

                    # broadcast mu, rsq to 128 partitions
                    ps_mu = ph_pool.tile([128, 1], F32, name="ps_mu", tag="ph")
                    nc.tensor.matmul(ps_mu, ones_row, mu, start=True, stop=True)
                    ps_rq = ph_pool.tile([128, 1], F32, name="ps_rq", tag="ph")
                    nc.tensor.matmul(ps_rq, ones_row, rsq, start=True, stop=True)
                    mu_bc = hs.tile([128, 1], F32, name="mu_bc")
                    nc.scalar.copy(out=mu_bc, in_=ps_mu)
                    rsq_bc = hs.tile([128, 1], F32, name="rsq_bc")
                    nc.scalar.copy(out=rsq_bc, in_=ps_rq)

                    z_sb = hs.tile([128, CB], F32, name="z_sb")
                    nc.vector.tensor_scalar(out=z_sb, in0=t_sb, scalar1=mu_bc,
                                            scalar2=rsq_bc, op0=ALU.subtract,
                                            op1=ALU.mult)
                    nc.vector.tensor_mul(z_sb, z_sb, lng_sb)
                    nc.vector.tensor_add(z_sb, z_sb, lnb_sb)
                    nc.vector.tensor_scalar_max(z_sb, z_sb, 0.0)   # relu

                    # t2 = cm2 @ relu + b ; gate = sigmoid(t2) via exp
                    ps_t2 = pht_pool.tile([128, CB], F32, name="ps_t2", tag="pht")
                    for cj in range(CB):
                        for mb in range(CB):
                            nc.tensor.matmul(
                                ps_t2[:, cj:cj + 1],
                                cm2t_sb[:, mb, cj * 128:(cj + 1) * 128],
                                z_sb[:, mb:mb + 1],
                                start=(mb == 0), stop=(mb == CB - 1))
                    s2t = hs.tile([128, CB], F32, name="s2t")
                    nc.vector.tensor_add(s2t, ps_t2, cm2b_sb)
                    ex = hs.tile([128, CB], F32, name="ex")
                    nc.scalar.activation(out=ex, in_=s2t, func=AF.Exp, scale=-1.0)
                    nc.vector.tensor_scalar_add(ex, ex, 1.0)
                    gate = hs.tile([128, CB], F32, name="gate")
                    nc.vector.reciprocal(out=gate, in_=ex)
                    scale_vec = hs.tile([128, CB], F32, name="scale_vec")
                    nc.vector.tensor_scalar_add(scale_vec, gate, 1.0)

                    finwts = hs.tile([128, CB, N], BF16, name="finwts")
                    for cb in range(CB):
                        nc.vector.tensor_scalar(
                            out=finwts[:, cb, :], in0=finwt_sb[:, cb, :],
                            scalar1=scale_vec[:, cb:cb + 1], scalar2=None,
                            op0=ALU.mult)

                # ---------------- final conv ----------------
                with contextlib.ExitStack() as fctx:
                    psF_pool = fctx.enter_context(
                        tc.tile_pool(name="psF", bufs=2, space="PSUM"))
                    PCW = 512
                    NPC = HW // PCW      # 32 chunks
                    DMAC = 4             # chunks per output DMA
                    for pc in range(NPC):
                        if pc % DMAC == 0:
                            outt = out_ring.tile([N, DMAC * PCW], F32, name="outt")
                        psF = psF_pool.tile([N, PCW], F32, name="psF")
                        for cb in range(CB):
                            nc.tensor.matmul(
                                psF, finwts[:, cb, :],
                                x_sb[:, cb, pc * PCW:(pc + 1) * PCW],
                                start=(cb == 0), stop=(cb == CB - 1))
                        dst = outt[:, (pc % DMAC) * PCW:(pc % DMAC + 1) * PCW]
                        if pc % 2 == 0:
                            nc.scalar.activation(out=dst, in_=psF, func=AF.Identity,
                                                 bias=finb_sb)
                        else:
                            nc.vector.tensor_scalar_add(dst, psF, finb_sb)
                        if pc % DMAC == DMAC - 1:
                            c0 = (pc - DMAC + 1) * PCW
                            nc.sync.dma_start(out=out_d.ap()[:, c0:c0 + DMAC * PCW],
                                              in_=outt)

            if repeat == 1:
                main_body()
            else:
                with tc.For_i(0, repeat, 1):
                    main_body()

    nc.compile()
    return nc


_cached = {}


def _get_module(repeat=1):
    if repeat not in _cached:
        _cached[repeat] = build_module(repeat)
    return _cached[repeat]


def prep_weights(inputs):
    f = np.float32
    map_w = np.asarray(inputs["map_w"], f)
    dist_w = np.asarray(inputs["dist_w"], f)
    bnd_w = np.asarray(inputs["bnd_w"], f)
    wt60 = np.zeros((C, 3 * FSLOT), f)
    wt60[:, 0:N] = map_w.T
    wt60[:, FSLOT:FSLOT + N] = dist_w.T
    wt60[:, 2 * FSLOT:2 * FSLOT + N] = bnd_w.T
    bias60 = np.zeros((3 * FSLOT,), f)
    bias60[0:N] = np.asarray(inputs["map_b"], f)
    bias60[FSLOT:FSLOT + N] = np.asarray(inputs["dist_b"], f)
    bias60[2 * FSLOT:2 * FSLOT + N] = np.asarray(inputs["bnd_b"], f)
    shared = {
        "wt60": wt60,
        "bias60": bias60,
        "maskw": np.asarray(inputs["mask_w"], f),
        "cm1T": np.ascontiguousarray(np.asarray(inputs["cm1_w"], f).T),
        "cm1b": np.asarray(inputs["cm1_b"], f),
        "lng": np.asarray(inputs["ln_g"], f),
        "lnb": np.asarray(inputs["ln_b"], f),
        "cm2T": np.ascontiguousarray(np.asarray(inputs["cm2_w"], f).T),
        "cm2b": np.asarray(inputs["cm2_b"], f),
        "finWT": np.ascontiguousarray(np.asarray(inputs["fin_w"], f).T),
        "finb": np.asarray(inputs["fin_b"], f),
    }
    return shared


def kernel(**inputs):
    nc = _get_module(1)
    shared = prep_weights(inputs)
    x = np.asarray(inputs["x"], np.float32)
    in_maps = []
    for b in range(B):
        m = dict(shared)
        m["x"] = np.ascontiguousarray(x[b].reshape(C, HW))
        in_maps.append(m)
    res = bass_utils.run_bass_kernel_spmd(nc, in_maps, core_ids=list(range(B)))
    out = np.stack([res.results[b]["out"].reshape(N, H, W) for b in range(B)])
    return out.astype(np.float32)
